# revision 7
# baseline (speedup 1.0000x reference)
"""Trainium2 Bass kernel for nn_AttentionScoreMask (topk_masking).

Per-core computation (batch sharded, one batch element per NeuronCore):
  Q^T = Wq @ q^T, K^T = Wk @ k^T           (PE, fp32 matmuls)
  per head: logits = Q_h^T' K_h / sqrt(hd)  (PE, fp32)
            e = exp(logits)                 (ScalarE, with row-sum accum)
            scores += e * (1/S_row)         (DVE fused multiply-add)
  top-512 per row, sorted desc with original indices:
    - per-row threshold tau via bisection so that count(s >= tau) in [512, ~528]
    - scatter-compact survivors to width W=576 (GPSIMD local_scatter, per-row)
    - 64x (max8 -> max_index -> match_replace) extraction on the compacted array
    - two more local_scatters invert the permutation into sorted original
      indices without any per-row gather
Host glue: shard across 8 cores, cast u16->int32, build the boolean row-0 mask.
"""
import sys

sys.path.insert(0, "/opt/trn_rl_repo")

import numpy as np
import concourse.bacc as bacc
import concourse.mybir as mybir
from concourse.tile import TileContext
from concourse.masks import make_identity
from concourse.bass_utils import run_bass_kernel_spmd

F32 = mybir.dt.float32
U16 = mybir.dt.uint16
I16 = mybir.dt.int16
U32 = mybir.dt.uint32
AF = mybir.ActivationFunctionType
ALU = mybir.AluOpType

B, N, C, H = 8, 1024, 768, 12
HD = C // H            # 64
K = 512                # top-k
NT = N // 128          # 8 row tiles
CT = C // 128          # 6 channel tiles
W = 576                # compacted width (>= max survivor count)
BIS = 6                # bisection rounds
ZBR = 0.35             # bisect bracket: mu +/- ZBR*sd
MU = 12.0 / N          # row mean of unnormalized scores (each head row sums to 1)
NEG = -1e30


def _build():
    nc = bacc.Bacc(None, target_bir_lowering=False)
    q_d = nc.declare_dram_parameter("q", [N, C], F32, isOutput=False)
    k_d = nc.declare_dram_parameter("k", [N, C], F32, isOutput=False)
    wq_d = nc.declare_dram_parameter("Wq", [C, C], F32, isOutput=False)
    wk_d = nc.declare_dram_parameter("Wk", [C, C], F32, isOutput=False)
    topk_d = nc.declare_dram_parameter("topk", [N, K], U16, isOutput=True)

    with TileContext(nc) as tc:
        with (
            tc.tile_pool(name="const", bufs=1) as cpool,
            tc.tile_pool(name="persist", bufs=1) as pp,
            tc.tile_pool(name="mpsum", bufs=2, space="PSUM") as mpsum,
            tc.tile_pool(name="small", bufs=4) as sm,
        ):
            ident = cpool.tile([128, 128], F32)
            make_identity(nc, ident[:])
            iota_pos = cpool.tile([128, N], U16)
            nc.gpsimd.iota(iota_pos[:], pattern=[[1, N]], base=0, channel_multiplier=0)
            iota_rk1 = cpool.tile([128, K], U16)
            nc.gpsimd.iota(iota_rk1[:], pattern=[[1, K]], base=1, channel_multiplier=0)

            QT = [pp.tile([128, N], F32, tag=f"QT{c}", name=f"QT{c}") for c in range(CT)]
            KT = [pp.tile([128, N], F32, tag=f"KT{c}", name=f"KT{c}") for c in range(CT)]

            with (
                tc.tile_pool(name="ph12", bufs=1) as p12,
                tc.tile_pool(name="io", bufs=3) as io,
                tc.tile_pool(name="tpsum", bufs=2, space="PSUM") as tpsum,
            ):
                qT = [p12.tile([128, N], F32, tag=f"qT{c}", name=f"qT{c}") for c in range(CT)]
                kT = [p12.tile([128, N], F32, tag=f"kT{c}", name=f"kT{c}") for c in range(CT)]
                wqT = [p12.tile([128, C], F32, tag=f"wqT{c}", name=f"wqT{c}") for c in range(CT)]
                wkT = [p12.tile([128, C], F32, tag=f"wkT{c}", name=f"wkT{c}") for c in range(CT)]

                # ---- phase 1: load + transpose q, k, Wq, Wk ----------------
                for src_d, dstT, nt in ((q_d, qT, NT), (k_d, kT, NT),
                                        (wq_d, wqT, CT), (wk_d, wkT, CT)):
                    for t in range(nt):
                        raw = io.tile([128, C], F32, tag="raw_in")
                        nc.sync.dma_start(raw[:], src_d[128 * t : 128 * (t + 1), :])
                        for c in range(CT):
                            ps = tpsum.tile([128, 128], F32, tag="tr")
                            nc.tensor.transpose(ps[:], raw[:, 128 * c : 128 * (c + 1)], ident[:])
                            nc.scalar.activation(
                                dstT[c][:, 128 * t : 128 * (t + 1)], ps[:], AF.Copy
                            )

                # ---- phase 2: projections  XT_out = W @ x^T ----------------
                for wT, xT, outT in ((wqT, qT, QT), (wkT, kT, KT)):
                    for m in range(CT):
                        for nchunk in range(2):
                            ps = mpsum.tile([128, 512], F32, tag="proj")
                            for kc in range(CT):
                                nc.tensor.matmul(
                                    ps[:],
                                    wT[kc][:, 128 * m : 128 * (m + 1)],
                                    xT[kc][:, 512 * nchunk : 512 * (nchunk + 1)],
                                    start=(kc == 0),
                                    stop=(kc == CT - 1),
                                )
                            nc.scalar.activation(
                                outT[m][:, 512 * nchunk : 512 * (nchunk + 1)], ps[:], AF.Copy
                            )

            # ---- phase 3+4 per row tile ------------------------------------
            scale = HD ** -0.5
            with tc.tile_pool(name="work", bufs=2) as wk:
                _phase34(nc, tc, wk, sm, mpsum, QT, KT, iota_pos, iota_rk1, topk_d, scale)
    nc.compile()
    return nc


def _phase34(nc, tc, wk, sm, mpsum, QT, KT, iota_pos, iota_rk1, topk_d, scale):
            for t in range(NT):
                scores = wk.tile([128, N], F32, tag="scores")
                nc.gpsimd.memset(scores[:], 0.0)
                for h in range(H):
                    ct, off = divmod(HD * h, 128)
                    ps = mpsum.tile([128, N], F32, tag="logits")
                    for nchunk in range(2):
                        nc.tensor.matmul(
                            ps[:, 512 * nchunk : 512 * (nchunk + 1)],
                            QT[ct][off : off + HD, 128 * t : 128 * (t + 1)],
                            KT[ct][off : off + HD, 512 * nchunk : 512 * (nchunk + 1)],
                        )
                    eh = wk.tile([128, N], F32, tag="exph")
                    sh = sm.tile([128, 1], F32, tag="sh")
                    nc.scalar.activation(eh[:], ps[:], AF.Exp, scale=scale, accum_out=sh[:])
                    rs = sm.tile([128, 1], F32, tag="rs")
                    nc.vector.reciprocal(rs[:], sh[:])
                    nc.vector.scalar_tensor_tensor(
                        scores[:], in0=eh[:], scalar=rs[:], in1=scores[:],
                        op0=ALU.mult, op1=ALU.add,
                    )

                # ---- topk on scores tile ----
                # sigma estimate
                sq = wk.tile([128, N], F32, tag="sqscr")
                m2 = sm.tile([128, 1], F32, tag="m2")
                nc.scalar.activation(sq[:], scores[:], AF.Square, accum_out=m2[:])
                var = sm.tile([128, 1], F32, tag="var")
                nc.vector.tensor_scalar(
                    out=var[:], in0=m2[:], scalar1=1.0 / N, scalar2=-MU * MU,
                    op0=ALU.mult, op1=ALU.add,
                )
                sd = sm.tile([128, 1], F32, tag="sd")
                nc.scalar.activation(sd[:], var[:], AF.Sqrt)
                lo = sm.tile([128, 1], F32, tag="lo")
                hi = sm.tile([128, 1], F32, tag="hi")
                nc.vector.tensor_scalar(
                    out=lo[:], in0=sd[:], scalar1=-ZBR, scalar2=MU, op0=ALU.mult, op1=ALU.add
                )
                nc.vector.tensor_scalar(
                    out=hi[:], in0=sd[:], scalar1=ZBR, scalar2=MU, op0=ALU.mult, op1=ALU.add
                )
                keep = wk.tile([128, N], F32, tag="keep")
                cnt = sm.tile([128, 1], F32, tag="cnt")
                mid = sm.tile([128, 1], F32, tag="mid")
                pred = sm.tile([128, 1], mybir.dt.uint8, tag="pred")
                for r in range(BIS):
                    nc.vector.tensor_tensor(out=mid[:], in0=lo[:], in1=hi[:], op=ALU.add)
                    nc.vector.tensor_scalar(out=mid[:], in0=mid[:], scalar1=0.5, scalar2=None, op0=ALU.mult)
                    nc.vector.tensor_scalar(
                        out=keep[:], in0=scores[:], scalar1=mid[:], scalar2=0.0,
                        op0=ALU.is_ge, op1=ALU.add, accum_out=cnt[:],
                    )
                    nc.vector.tensor_scalar(
                        out=pred[:], in0=cnt[:], scalar1=512.5, scalar2=None, op0=ALU.is_ge
                    )
                    nc.vector.copy_predicated(out=lo[:], mask=pred[:], data=mid[:])
                    nc.vector.tensor_scalar(
                        out=pred[:], in0=pred[:], scalar1=1.0, scalar2=None, op0=ALU.is_lt
                    )
                    nc.vector.copy_predicated(out=hi[:], mask=pred[:], data=mid[:])
                # final keep mask + compaction positions
                nc.vector.tensor_scalar(
                    out=keep[:], in0=scores[:], scalar1=lo[:], scalar2=None, op0=ALU.is_ge
                )
                pos = wk.tile([128, N], F32, tag="pos")
                nc.vector.tensor_tensor_scan(
                    out=pos[:], data0=keep[:], data1=keep[:], initial=0.0,
                    op0=ALU.add, op1=ALU.bypass,
                )
                nc.vector.tensor_tensor(out=pos[:], in0=pos[:], in1=keep[:], op=ALU.mult)
                nc.vector.tensor_scalar(
                    out=pos[:], in0=pos[:], scalar1=1.0, scalar2=None, op0=ALU.subtract
                )
                sidx = wk.tile([128, N], I16, tag="sidx")
                nc.vector.tensor_copy(sidx[:], pos[:])
                # value halves as u16 planes
                s16 = scores[:].bitcast(U16).rearrange("p (n two) -> p n two", two=2)
                vlo = wk.tile([128, N], U16, tag="vlo")
                vhi = wk.tile([128, N], U16, tag="vhi")
                nc.vector.tensor_copy(vlo[:], s16[:, :, 0])
                nc.vector.tensor_copy(vhi[:], s16[:, :, 1])
                cmap = wk.tile([128, W], U16, tag="cmap")
                clo = wk.tile([128, W], U16, tag="clo")
                chi = wk.tile([128, W], U16, tag="chi")
                nc.gpsimd.local_scatter(cmap[:], iota_pos[:], sidx[:], channels=128, num_elems=W, num_idxs=N)
                nc.gpsimd.local_scatter(clo[:], vlo[:], sidx[:], channels=128, num_elems=W, num_idxs=N)
                nc.gpsimd.local_scatter(chi[:], vhi[:], sidx[:], channels=128, num_elems=W, num_idxs=N)
                comp = wk.tile([128, W], F32, tag="comp")
                c16 = comp[:].bitcast(U16).rearrange("p (n two) -> p n two", two=2)
                nc.vector.tensor_copy(c16[:, :, 0], clo[:])
                nc.vector.tensor_copy(c16[:, :, 1], chi[:])
                # extraction: 64 rounds of top-8
                mx = sm.tile([128, 8], F32, tag="mx")
                mi = wk.tile([128, K], U16, tag="mi")
                for it in range(K // 8):
                    nc.vector.max(out=mx[:], in_=comp[:])
                    nc.vector.max_index(out=mi[:, 8 * it : 8 * (it + 1)], in_max=mx[:], in_values=comp[:])
                    nc.vector.match_replace(out=comp[:], in_to_replace=mx[:], in_values=comp[:], imm_value=NEG)
                # invert permutation: rank-of-compacted, then original index per rank
                rk1 = wk.tile([128, W], U16, tag="rk1")
                nc.gpsimd.local_scatter(rk1[:], iota_rk1[:], mi[:].bitcast(I16), channels=128, num_elems=W, num_idxs=K)
                rk = wk.tile([128, W], I16, tag="rk")
                nc.vector.tensor_scalar(
                    out=rk[:], in0=rk1[:], scalar1=1.0, scalar2=None, op0=ALU.subtract
                )
                oidx = wk.tile([128, K], U16, tag="oidx")
                nc.gpsimd.local_scatter(oidx[:], cmap[:], rk[:], channels=128, num_elems=K, num_idxs=W)
                nc.sync.dma_start(topk_d[128 * t : 128 * (t + 1), :], oidx[:])


_NC = None


def kernel(q, k, Wq, Wk):
    global _NC
    if _NC is None:
        _NC = _build()
    in_maps = [
        {
            "q": np.ascontiguousarray(q[b], dtype=np.float32),
            "k": np.ascontiguousarray(k[b], dtype=np.float32),
            "Wq": np.ascontiguousarray(Wq, dtype=np.float32),
            "Wk": np.ascontiguousarray(Wk, dtype=np.float32),
        }
        for b in range(B)
    ]
    res = run_bass_kernel_spmd(_NC, in_maps, list(range(B))).results
    topk = np.stack([res[b]["topk"].astype(np.int32) for b in range(B)])
    mask = np.zeros((B, N), dtype=np.bool_)
    for b in range(B):
        mask[b, topk[b, 0]] = True
    return mask, topk


# revision 26
# speedup vs baseline: 1.4572x; 1.4572x over previous
"""Trainium2 Bass kernel for nn_AttentionScoreMask (topk_masking).

Per-core computation (batch sharded, one batch element per NeuronCore):
  Q^T = Wq @ q^T, K^T = Wk @ k^T           (PE, fp32 matmuls)
  per head: logits = Q_h^T' K_h / sqrt(hd)  (PE, fp32)
            e = exp(logits)                 (ScalarE, with row-sum accum)
            scores += e * (1/S_row)         (DVE fused multiply-add)
  top-512 per row, sorted desc with original indices:
    - per-row threshold tau via bisection so that count(s >= tau) in [512, ~528]
    - scatter-compact survivors to width W=576 (GPSIMD local_scatter, per-row)
    - 64x (max8 -> max_index -> match_replace) extraction on the compacted array
    - two more local_scatters invert the permutation into sorted original
      indices without any per-row gather
Host glue: shard across 8 cores, cast u16->int32, build the boolean row-0 mask.
"""
import sys

sys.path.insert(0, "/opt/trn_rl_repo")

import numpy as np
import concourse.bacc as bacc
import concourse.mybir as mybir
from concourse.tile import TileContext
from concourse.masks import make_identity
from concourse.bass_utils import run_bass_kernel_spmd

F32 = mybir.dt.float32
U16 = mybir.dt.uint16
I16 = mybir.dt.int16
U32 = mybir.dt.uint32
AF = mybir.ActivationFunctionType
ALU = mybir.AluOpType

B, N, C, H = 8, 1024, 768, 12
HD = C // H            # 64
K = 512                # top-k
NT = N // 128          # 8 row tiles
CT = C // 128          # 6 channel tiles
W = 704                # segmented width: 256 + 224 + 224
BIS = 6                # bisection rounds
ZBR = 0.35             # bisect bracket: mu +/- ZBR*sd
MU = 12.0 / N          # row mean of unnormalized scores (each head row sums to 1)
NEG = -1e30


def _build():
    nc = bacc.Bacc(None, target_bir_lowering=False)
    q_d = nc.declare_dram_parameter("q", [N, C], F32, isOutput=False)
    k_d = nc.declare_dram_parameter("k", [N, C], F32, isOutput=False)
    wq_d = nc.declare_dram_parameter("Wq", [C, C], F32, isOutput=False)
    wk_d = nc.declare_dram_parameter("Wk", [C, C], F32, isOutput=False)
    topk_d = nc.declare_dram_parameter("topk", [N, K], U16, isOutput=True)

    with TileContext(nc) as tc:
        with (
            tc.tile_pool(name="const", bufs=1) as cpool,
            tc.tile_pool(name="persist", bufs=1) as pp,
            tc.tile_pool(name="small", bufs=4) as sm,
        ):
            ident = cpool.tile([128, 128], F32)
            make_identity(nc, ident[:])
            iota_pos = cpool.tile([128, N], U16)
            nc.gpsimd.iota(iota_pos[:], pattern=[[1, N]], base=0, channel_multiplier=0)
            iota_rk1 = cpool.tile([128, K], U16)
            nc.gpsimd.iota(iota_rk1[:], pattern=[[1, K]], base=1, channel_multiplier=0)

            QT = [pp.tile([128, N], F32, tag=f"QT{c}", name=f"QT{c}") for c in range(CT)]
            KT = [pp.tile([128, N], F32, tag=f"KT{c}", name=f"KT{c}") for c in range(CT)]

            with (
                tc.tile_pool(name="ph12", bufs=1) as p12,
                tc.tile_pool(name="io", bufs=3) as io,
                tc.tile_pool(name="tpsum", bufs=2, space="PSUM") as tpsum,
            ):
                qT = [p12.tile([128, N], F32, tag=f"qT{c}", name=f"qT{c}") for c in range(CT)]
                kT = [p12.tile([128, N], F32, tag=f"kT{c}", name=f"kT{c}") for c in range(CT)]
                wqT = [p12.tile([128, C], F32, tag=f"wqT{c}", name=f"wqT{c}") for c in range(CT)]
                wkT = [p12.tile([128, C], F32, tag=f"wkT{c}", name=f"wkT{c}") for c in range(CT)]

                # ---- phase 1: load + transpose q, k, Wq, Wk ----------------
                for src_d, dstT, nt in ((q_d, qT, NT), (k_d, kT, NT),
                                        (wq_d, wqT, CT), (wk_d, wkT, CT)):
                    for t in range(nt):
                        raw = io.tile([128, C], F32, tag="raw_in")
                        nc.sync.dma_start(raw[:], src_d[128 * t : 128 * (t + 1), :])
                        for c in range(CT):
                            ps = tpsum.tile([128, 128], F32, tag="tr")
                            nc.tensor.transpose(ps[:], raw[:, 128 * c : 128 * (c + 1)], ident[:])
                            nc.vector.tensor_copy(
                                dstT[c][:, 128 * t : 128 * (t + 1)], ps[:]
                            )

                # ---- phase 2: projections  XT_out = W @ x^T ----------------
                for m in range(CT):
                    for wT, xT, outT in ((wqT, qT, QT), (wkT, kT, KT)):
                        for nchunk in range(2):
                            ps = tpsum.tile([128, 512], F32, tag="proj")
                            for kc in range(CT):
                                nc.tensor.matmul(
                                    ps[:],
                                    wT[kc][:, 128 * m : 128 * (m + 1)],
                                    xT[kc][:, 512 * nchunk : 512 * (nchunk + 1)],
                                    start=(kc == 0),
                                    stop=(kc == CT - 1),
                                )
                            nc.vector.tensor_copy(
                                outT[m][:, 512 * nchunk : 512 * (nchunk + 1)], ps[:]
                            )

            # ---- phase 3+4 per row tile ------------------------------------
            scale = HD ** -0.5
            with (
                tc.tile_pool(name="work", bufs=2) as wk,
                tc.tile_pool(name="mpsum", bufs=2, space="PSUM") as mpsum,
            ):
                _phase34(nc, tc, wk, sm, mpsum, QT, KT, iota_pos, iota_rk1, topk_d, scale, ident)
    nc.compile()
    return nc


def _phase34(nc, tc, wk, sm, mpsum, QT, KT, iota_pos, iota_rk1, topk_d, scale, ident):
    scores = {}
    state = {}
    # value-ordered segments: [mu+0.8sd, inf) -> cols [0,256);
    # [mu+0.32sd, mu+0.8sd) -> [256,480); [tau_bisect, mu+0.32sd) -> [480,704).
    # Segment caps hold the offline-calibrated per-row count ranges with margin.
    Z1, Z2 = 0.8, 0.32
    BASES = (0, 256, 480)
    CAPS = (256, 224, 224)
    ITERS = (32, 28, 24)

    def softmax(t, pre_ops=None):
        s = wk.tile([128, N], F32, tag=f"scores{t % 4}", name=f"scores_{t}", bufs=1)
        scores[t] = s
        nc.gpsimd.memset(s[:], 0.0)
        for h in range(H):
            if pre_ops is not None and h in pre_ops:
                pre_ops[h]()  # staged topk work for the previous tile
            ct, off = divmod(HD * h, 128)
            ps = mpsum.tile([128, N], F32, tag="logits", bufs=4)
            for nchunk in range(2):
                nc.tensor.matmul(
                    ps[:, 512 * nchunk : 512 * (nchunk + 1)],
                    QT[ct][off : off + HD, 128 * t : 128 * (t + 1)],
                    KT[ct][off : off + HD, 512 * nchunk : 512 * (nchunk + 1)],
                )
            eh = wk.tile([128, N], F32, tag="exph", bufs=4)
            sh = sm.tile([128, 1], F32, tag="sh")
            nc.scalar.activation(eh[:], ps[:], AF.Exp, scale=scale, accum_out=sh[:])
            rs = sm.tile([128, 1], F32, tag="rs")
            nc.vector.reciprocal(rs[:], sh[:])
            nc.vector.scalar_tensor_tensor(
                s[:], in0=eh[:], scalar=rs[:], in1=s[:],
                op0=ALU.mult, op1=ALU.add,
            )

    def stage_a(t):
        s = scores[t]
        sq = wk.tile([128, N], F32, tag="sqscr")
        m2 = sm.tile([128, 1], F32, tag="m2")
        nc.scalar.activation(sq[:], s[:], AF.Square, accum_out=m2[:])
        var = sm.tile([128, 1], F32, tag="var")
        nc.vector.tensor_scalar(
            out=var[:], in0=m2[:], scalar1=1.0 / N, scalar2=-MU * MU,
            op0=ALU.mult, op1=ALU.add,
        )
        sd = sm.tile([128, 1], F32, tag="sd")
        nc.scalar.activation(sd[:], var[:], AF.Sqrt)
        lo = sm.tile([128, 1], F32, tag="lo")
        hi = sm.tile([128, 1], F32, tag="hi")
        tau1 = sm.tile([128, 1], F32, tag="tau1")
        tau2 = sm.tile([128, 1], F32, tag="tau2")
        nc.vector.tensor_scalar(out=lo[:], in0=sd[:], scalar1=-ZBR, scalar2=MU, op0=ALU.mult, op1=ALU.add)
        nc.vector.tensor_scalar(out=hi[:], in0=sd[:], scalar1=ZBR, scalar2=MU, op0=ALU.mult, op1=ALU.add)
        nc.vector.tensor_scalar(out=tau1[:], in0=sd[:], scalar1=Z1, scalar2=MU, op0=ALU.mult, op1=ALU.add)
        nc.vector.tensor_scalar(out=tau2[:], in0=sd[:], scalar1=Z2, scalar2=MU, op0=ALU.mult, op1=ALU.add)
        ka = wk.tile([128, N], F32, tag="ka")
        cnt = sm.tile([128, 1], F32, tag="cnt")
        mid = sm.tile([128, 1], F32, tag="mid")
        pred = sm.tile([128, 1], mybir.dt.uint8, tag="pred")
        nmid = sm.tile([128, 1], F32, tag="nmid")
        for r in range(BIS):
            nc.vector.tensor_tensor(out=mid[:], in0=lo[:], in1=hi[:], op=ALU.add)
            nc.vector.tensor_scalar(out=mid[:], in0=mid[:], scalar1=0.5, scalar2=None, op0=ALU.mult)
            nc.vector.tensor_scalar(out=nmid[:], in0=mid[:], scalar1=-1.0, scalar2=None, op0=ALU.mult)
            # cnt' = #(s>mid) - #(s<mid) = 2*count_ge - 1024 (+-ties); count>=513 <=> cnt'>=2
            nc.scalar.activation(ka[:], s[:], AF.Sign, bias=nmid[:], accum_out=cnt[:])
            nc.vector.tensor_scalar(out=pred[:], in0=cnt[:], scalar1=1.5, scalar2=None, op0=ALU.is_ge)
            nc.vector.copy_predicated(out=lo[:], mask=pred[:], data=mid[:])
            nc.vector.tensor_scalar(out=pred[:], in0=pred[:], scalar1=1.0, scalar2=None, op0=ALU.is_lt)
            nc.vector.copy_predicated(out=hi[:], mask=pred[:], data=mid[:])
        # segment masks + exact counts c1, c2
        c1f = sm.tile([128, 1], F32, tag="c1f", name=f"c1f_{t}", bufs=2)
        c2f = sm.tile([128, 1], F32, tag="c2f", name=f"c2f_{t}", bufs=2)
        kb = wk.tile([128, N], F32, tag="kb")
        kc = wk.tile([128, N], F32, tag="kc")
        nc.vector.tensor_scalar(
            out=ka[:], in0=s[:], scalar1=tau1[:], scalar2=0.0,
            op0=ALU.is_ge, op1=ALU.add, accum_out=c1f[:],
        )
        nc.vector.tensor_scalar(
            out=kb[:], in0=s[:], scalar1=tau2[:], scalar2=0.0,
            op0=ALU.is_ge, op1=ALU.add, accum_out=c2f[:],
        )
        nc.vector.tensor_scalar(out=kc[:], in0=s[:], scalar1=lo[:], scalar2=None, op0=ALU.is_ge)
        # exclusive masks (overwrite kb, kc)
        nc.vector.tensor_tensor(out=kc[:], in0=kc[:], in1=kb[:], op=ALU.subtract)
        nc.vector.tensor_tensor(out=kb[:], in0=kb[:], in1=ka[:], op=ALU.subtract)
        # per-segment stable positions
        sa = wk.tile([128, N], F32, tag="sa")
        sb = wk.tile([128, N], F32, tag="sb")
        sc = wk.tile([128, N], F32, tag="sc")
        nc.vector.tensor_tensor_scan(out=sa[:], data0=ka[:], data1=ka[:], initial=0.0, op0=ALU.add, op1=ALU.bypass)
        nc.vector.tensor_tensor_scan(out=sb[:], data0=kb[:], data1=kb[:], initial=0.0, op0=ALU.add, op1=ALU.bypass)
        nc.vector.tensor_tensor_scan(out=sc[:], data0=kc[:], data1=kc[:], initial=0.0, op0=ALU.add, op1=ALU.bypass)
        # pos = ka*sa + kb*(sb+256) + kc*(sc+480) - 1, clamped to W-1
        nc.vector.tensor_tensor(out=sa[:], in0=sa[:], in1=ka[:], op=ALU.mult)
        nc.vector.scalar_tensor_tensor(out=sb[:], in0=sb[:], scalar=float(BASES[1]), in1=kb[:], op0=ALU.add, op1=ALU.mult)
        nc.vector.scalar_tensor_tensor(out=sc[:], in0=sc[:], scalar=float(BASES[2]), in1=kc[:], op0=ALU.add, op1=ALU.mult)
        nc.vector.tensor_tensor(out=sa[:], in0=sa[:], in1=sb[:], op=ALU.add)
        nc.vector.tensor_tensor(out=sa[:], in0=sa[:], in1=sc[:], op=ALU.add)
        nc.vector.tensor_scalar(
            out=sa[:], in0=sa[:], scalar1=1.0, scalar2=float(W - 1),
            op0=ALU.subtract, op1=ALU.min,
        )
        sidx = wk.tile([128, N], I16, tag="sidx", name=f"sidx_{t}", bufs=2)
        nc.vector.tensor_copy(sidx[:], sa[:])
        s16 = s[:].bitcast(U16).rearrange("p (n two) -> p n two", two=2)
        vlo = wk.tile([128, N], U16, tag="vlo", name=f"vlo_{t}", bufs=2)
        vhi = wk.tile([128, N], U16, tag="vhi", name=f"vhi_{t}", bufs=2)
        nc.scalar.activation(vlo[:], s16[:, :, 0], AF.Copy)
        nc.scalar.activation(vhi[:], s16[:, :, 1], AF.Copy)
        state[t] = (sidx, vlo, vhi, c1f, c2f)

    def stage_b(t):
        sidx, vlo, vhi, c1f, c2f = state[t]
        cmap = wk.tile([128, W], U16, tag="cmap", name=f"cmap_{t}", bufs=2)
        clo = wk.tile([128, W], U16, tag="clo", name=f"clo_{t}", bufs=2)
        chi = wk.tile([128, W], U16, tag="chi", name=f"chi_{t}", bufs=2)
        nc.gpsimd.local_scatter(cmap[:], iota_pos[:], sidx[:], channels=128, num_elems=W, num_idxs=N)
        nc.gpsimd.local_scatter(clo[:], vlo[:], sidx[:], channels=128, num_elems=W, num_idxs=N)
        nc.gpsimd.local_scatter(chi[:], vhi[:], sidx[:], channels=128, num_elems=W, num_idxs=N)
        state[t] = (cmap, clo, chi, c1f, c2f)

    def stage_c(t, seg):
        if seg == 0:
            cmap, clo, chi, c1f, c2f = state[t]
            comp = wk.tile([128, W], F32, tag="comp", name=f"comp_{t}")
            c16 = comp[:].bitcast(U16).rearrange("p (n two) -> p n two", two=2)
            nc.vector.tensor_copy(c16[:, :, 0], clo[:])
            nc.vector.tensor_copy(c16[:, :, 1], chi[:])
            mi = wk.tile([128, 8 * sum(ITERS)], U16, tag="mi", name=f"mi_{t}", bufs=2)
            state[t] = (cmap, chi, mi, c1f, c2f, comp)
        cmap, chi, mi, c1f, c2f, comp = state[t]
        mx = sm.tile([128, 8], F32, tag="mx")
        col = 8 * sum(ITERS[:seg])
        cc = comp[:, BASES[seg] : BASES[seg] + CAPS[seg]]
        for _ in range(ITERS[seg]):
            nc.vector.max(out=mx[:], in_=cc)
            nc.vector.max_index(out=mi[:, col : col + 8], in_max=mx[:], in_values=cc)
            nc.vector.match_replace(out=cc, in_to_replace=mx[:], in_values=cc, imm_value=NEG)
            col += 8

    def stage_d(t):
        cmap, chi, mi, c1f, c2f, _comp = state[t]
        # rank+1 data per segment (global rank = segment count offset + order)
        rk1 = wk.tile([128, W], U16, tag="rk1", name=f"rk1_{t}")
        col = 0
        for seg, (base, cap, its) in enumerate(zip(BASES, CAPS, ITERS)):
            ln = 8 * its
            if seg == 0:
                rdata = iota_rk1[:, :ln]
            else:
                off = c1f if seg == 1 else c2f
                rd = wk.tile([128, ln], U16, tag=f"rd{seg}", name=f"rd{seg}_{t}")
                nc.vector.tensor_scalar(out=rd[:], in0=iota_rk1[:, :ln], scalar1=off[:], scalar2=None, op0=ALU.add)
                rdata = rd[:]
            nc.gpsimd.local_scatter(
                rk1[:, base : base + cap], rdata, mi[:, col : col + ln].bitcast(I16),
                channels=128, num_elems=cap, num_idxs=ln,
            )
            col += 8 * its
        # kill garbage slots (zero-padding) and ranks > 512, then invert
        g = wk.tile([128, W], F32, tag="gmask", name=f"g_{t}")
        nc.vector.tensor_scalar(out=g[:], in0=chi[:], scalar1=0.5, scalar2=None, op0=ALU.is_gt)
        rf = wk.tile([128, W], F32, tag="rf", name=f"rf_{t}")
        nc.vector.tensor_tensor(out=rf[:], in0=rk1[:], in1=g[:], op=ALU.mult)
        nc.vector.tensor_scalar(out=g[:], in0=rf[:], scalar1=512.5, scalar2=None, op0=ALU.is_lt)
        nc.vector.tensor_tensor(out=rf[:], in0=rf[:], in1=g[:], op=ALU.mult)
        rk = wk.tile([128, W], I16, tag="rk", name=f"rk_{t}")
        nc.vector.tensor_scalar(out=rk[:], in0=rf[:], scalar1=1.0, scalar2=None, op0=ALU.subtract)
        oidx = wk.tile([128, K], U16, tag="oidx", name=f"oidx_{t}")
        nc.gpsimd.local_scatter(oidx[:], cmap[:], rk[:], channels=128, num_elems=K, num_idxs=W)
        nc.sync.dma_start(topk_d[128 * t : 128 * (t + 1), :], oidx[:])
        del state[t], scores[t]

    def make_pre(t):
        return {
            0: lambda: (stage_a(t), stage_b(t)),
            5: lambda: stage_c(t, 0),
            8: lambda: stage_c(t, 1),
            10: lambda: (stage_c(t, 2), stage_d(t)),
        }

    for t in range(NT):
        softmax(t, pre_ops=make_pre(t - 1) if t >= 1 else None)
    stage_a(NT - 1)
    stage_b(NT - 1)
    for seg in range(3):
        stage_c(NT - 1, seg)
    stage_d(NT - 1)





# revision 27
# speedup vs baseline: 1.4634x; 1.0042x over previous
"""Trainium2 Bass kernel for nn_AttentionScoreMask (topk_masking).

Per-core computation (batch sharded, one batch element per NeuronCore):
  Q^T = Wq @ q^T, K^T = Wk @ k^T           (PE, fp32 matmuls)
  per head: logits = Q_h^T' K_h / sqrt(hd)  (PE, fp32)
            e = exp(logits)                 (ScalarE, with row-sum accum)
            scores += e * (1/S_row)         (DVE fused multiply-add)
  top-512 per row, sorted desc with original indices:
    - per-row threshold tau via bisection so that count(s >= tau) in [512, ~528]
    - scatter-compact survivors to width W=576 (GPSIMD local_scatter, per-row)
    - 64x (max8 -> max_index -> match_replace) extraction on the compacted array
    - two more local_scatters invert the permutation into sorted original
      indices without any per-row gather
Host glue: shard across 8 cores, cast u16->int32, build the boolean row-0 mask.
"""
import sys

sys.path.insert(0, "/opt/trn_rl_repo")

import numpy as np
import concourse.bacc as bacc
import concourse.mybir as mybir
from concourse.tile import TileContext
from concourse.masks import make_identity
from concourse.bass_utils import run_bass_kernel_spmd

F32 = mybir.dt.float32
U16 = mybir.dt.uint16
I16 = mybir.dt.int16
U32 = mybir.dt.uint32
AF = mybir.ActivationFunctionType
ALU = mybir.AluOpType

B, N, C, H = 8, 1024, 768, 12
HD = C // H            # 64
K = 512                # top-k
NT = N // 128          # 8 row tiles
CT = C // 128          # 6 channel tiles
W = 704                # segmented width: 256 + 224 + 224
BIS = 6                # bisection rounds
ZBR = 0.35             # bisect bracket: mu +/- ZBR*sd
MU = 12.0 / N          # row mean of unnormalized scores (each head row sums to 1)
NEG = -1e30


def _build():
    nc = bacc.Bacc(None, target_bir_lowering=False)
    q_d = nc.declare_dram_parameter("q", [N, C], F32, isOutput=False)
    k_d = nc.declare_dram_parameter("k", [N, C], F32, isOutput=False)
    wq_d = nc.declare_dram_parameter("Wq", [C, C], F32, isOutput=False)
    wk_d = nc.declare_dram_parameter("Wk", [C, C], F32, isOutput=False)
    topk_d = nc.declare_dram_parameter("topk", [N, K], U16, isOutput=True)

    with TileContext(nc) as tc:
        with (
            tc.tile_pool(name="const", bufs=1) as cpool,
            tc.tile_pool(name="persist", bufs=1) as pp,
            tc.tile_pool(name="small", bufs=4) as sm,
        ):
            ident = cpool.tile([128, 128], F32)
            make_identity(nc, ident[:])
            iota_pos = cpool.tile([128, N], U16)
            nc.gpsimd.iota(iota_pos[:], pattern=[[1, N]], base=0, channel_multiplier=0)
            iota_rk1 = cpool.tile([128, K], U16)
            nc.gpsimd.iota(iota_rk1[:], pattern=[[1, K]], base=1, channel_multiplier=0)

            QT = [pp.tile([128, N], F32, tag=f"QT{c}", name=f"QT{c}") for c in range(CT)]
            KT = [pp.tile([128, N], F32, tag=f"KT{c}", name=f"KT{c}") for c in range(CT)]

            with (
                tc.tile_pool(name="ph12", bufs=1) as p12,
                tc.tile_pool(name="io", bufs=3) as io,
                tc.tile_pool(name="tpsum", bufs=2, space="PSUM") as tpsum,
            ):
                qT = [p12.tile([128, N], F32, tag=f"qT{c}", name=f"qT{c}") for c in range(CT)]
                kT = [p12.tile([128, N], F32, tag=f"kT{c}", name=f"kT{c}") for c in range(CT)]
                wqT = [p12.tile([128, C], F32, tag=f"wqT{c}", name=f"wqT{c}") for c in range(CT)]
                wkT = [p12.tile([128, C], F32, tag=f"wkT{c}", name=f"wkT{c}") for c in range(CT)]

                # ---- phase 1: load + transpose q, k, Wq, Wk ----------------
                for src_d, dstT, nt in ((q_d, qT, NT), (k_d, kT, NT),
                                        (wq_d, wqT, CT), (wk_d, wkT, CT)):
                    for t in range(nt):
                        raw = io.tile([128, C], F32, tag="raw_in")
                        nc.sync.dma_start(raw[:], src_d[128 * t : 128 * (t + 1), :])
                        for c in range(CT):
                            ps = tpsum.tile([128, 128], F32, tag="tr")
                            nc.tensor.transpose(ps[:], raw[:, 128 * c : 128 * (c + 1)], ident[:])
                            nc.vector.tensor_copy(
                                dstT[c][:, 128 * t : 128 * (t + 1)], ps[:]
                            )

                # ---- phase 2: projections  XT_out = W @ x^T ----------------
                for m in range(CT):
                    for wT, xT, outT in ((wqT, qT, QT), (wkT, kT, KT)):
                        for nchunk in range(2):
                            ps = tpsum.tile([128, 512], F32, tag="proj")
                            for kc in range(CT):
                                nc.tensor.matmul(
                                    ps[:],
                                    wT[kc][:, 128 * m : 128 * (m + 1)],
                                    xT[kc][:, 512 * nchunk : 512 * (nchunk + 1)],
                                    start=(kc == 0),
                                    stop=(kc == CT - 1),
                                )
                            nc.vector.tensor_copy(
                                outT[m][:, 512 * nchunk : 512 * (nchunk + 1)], ps[:]
                            )

            # ---- phase 3+4 per row tile ------------------------------------
            scale = HD ** -0.5
            with (
                tc.tile_pool(name="work", bufs=2) as wk,
                tc.tile_pool(name="mpsum", bufs=2, space="PSUM") as mpsum,
            ):
                _phase34(nc, tc, wk, sm, mpsum, QT, KT, iota_pos, iota_rk1, topk_d, scale, ident)
    nc.compile()
    return nc


def _phase34(nc, tc, wk, sm, mpsum, QT, KT, iota_pos, iota_rk1, topk_d, scale, ident):
    scores = {}
    state = {}
    # value-ordered segments: [mu+0.8sd, inf) -> cols [0,256);
    # [mu+0.32sd, mu+0.8sd) -> [256,480); [tau_bisect, mu+0.32sd) -> [480,704).
    # Segment caps hold the offline-calibrated per-row count ranges with margin.
    Z1, Z2 = 0.8, 0.32
    BASES = (0, 256, 480)
    CAPS = (256, 224, 224)
    ITERS = (32, 28, 24)

    def softmax(t, pre_ops=None):
        s = wk.tile([128, N], F32, tag=f"scores{t % 4}", name=f"scores_{t}", bufs=1)
        scores[t] = s
        nc.gpsimd.memset(s[:], 0.0)
        for h in range(H):
            if pre_ops is not None and h in pre_ops:
                pre_ops[h]()  # staged topk work for the previous tile
            ct, off = divmod(HD * h, 128)
            ps = mpsum.tile([128, N], F32, tag="logits", bufs=3)
            for nchunk in range(2):
                nc.tensor.matmul(
                    ps[:, 512 * nchunk : 512 * (nchunk + 1)],
                    QT[ct][off : off + HD, 128 * t : 128 * (t + 1)],
                    KT[ct][off : off + HD, 512 * nchunk : 512 * (nchunk + 1)],
                )
            eh = wk.tile([128, N], F32, tag="exph", bufs=4)
            sh = sm.tile([128, 1], F32, tag="sh")
            nc.scalar.activation(eh[:], ps[:], AF.Exp, scale=scale, accum_out=sh[:])
            rs = sm.tile([128, 1], F32, tag="rs")
            nc.vector.reciprocal(rs[:], sh[:])
            nc.vector.scalar_tensor_tensor(
                s[:], in0=eh[:], scalar=rs[:], in1=s[:],
                op0=ALU.mult, op1=ALU.add,
            )

    def stage_a(t):
        s = scores[t]
        sq = wk.tile([128, N], F32, tag="sqscr")
        m2 = sm.tile([128, 1], F32, tag="m2")
        nc.scalar.activation(sq[:], s[:], AF.Square, accum_out=m2[:])
        var = sm.tile([128, 1], F32, tag="var")
        nc.vector.tensor_scalar(
            out=var[:], in0=m2[:], scalar1=1.0 / N, scalar2=-MU * MU,
            op0=ALU.mult, op1=ALU.add,
        )
        sd = sm.tile([128, 1], F32, tag="sd")
        nc.scalar.activation(sd[:], var[:], AF.Sqrt)
        lo = sm.tile([128, 1], F32, tag="lo")
        hi = sm.tile([128, 1], F32, tag="hi")
        tau1 = sm.tile([128, 1], F32, tag="tau1")
        tau2 = sm.tile([128, 1], F32, tag="tau2")
        nc.vector.tensor_scalar(out=lo[:], in0=sd[:], scalar1=-ZBR, scalar2=MU, op0=ALU.mult, op1=ALU.add)
        nc.vector.tensor_scalar(out=hi[:], in0=sd[:], scalar1=ZBR, scalar2=MU, op0=ALU.mult, op1=ALU.add)
        nc.vector.tensor_scalar(out=tau1[:], in0=sd[:], scalar1=Z1, scalar2=MU, op0=ALU.mult, op1=ALU.add)
        nc.vector.tensor_scalar(out=tau2[:], in0=sd[:], scalar1=Z2, scalar2=MU, op0=ALU.mult, op1=ALU.add)
        ka = wk.tile([128, N], F32, tag="ka")
        cnt = sm.tile([128, 1], F32, tag="cnt")
        mid = sm.tile([128, 1], F32, tag="mid")
        pred = sm.tile([128, 1], mybir.dt.uint8, tag="pred")
        nmid = sm.tile([128, 1], F32, tag="nmid")
        for r in range(BIS):
            nc.vector.tensor_tensor(out=mid[:], in0=lo[:], in1=hi[:], op=ALU.add)
            nc.vector.tensor_scalar(out=mid[:], in0=mid[:], scalar1=0.5, scalar2=None, op0=ALU.mult)
            nc.vector.tensor_scalar(out=nmid[:], in0=mid[:], scalar1=-1.0, scalar2=None, op0=ALU.mult)
            # cnt' = #(s>mid) - #(s<mid) = 2*count_ge - 1024 (+-ties); count>=513 <=> cnt'>=2
            nc.scalar.activation(ka[:], s[:], AF.Sign, bias=nmid[:], accum_out=cnt[:])
            nc.vector.tensor_scalar(out=pred[:], in0=cnt[:], scalar1=1.5, scalar2=None, op0=ALU.is_ge)
            nc.vector.copy_predicated(out=lo[:], mask=pred[:], data=mid[:])
            nc.vector.tensor_scalar(out=pred[:], in0=pred[:], scalar1=1.0, scalar2=None, op0=ALU.is_lt)
            nc.vector.copy_predicated(out=hi[:], mask=pred[:], data=mid[:])
        # segment masks + exact counts c1, c2
        c1f = sm.tile([128, 1], F32, tag="c1f", name=f"c1f_{t}", bufs=2)
        c2f = sm.tile([128, 1], F32, tag="c2f", name=f"c2f_{t}", bufs=2)
        kb = wk.tile([128, N], F32, tag="kb")
        kc = wk.tile([128, N], F32, tag="kc")
        nc.vector.tensor_scalar(
            out=ka[:], in0=s[:], scalar1=tau1[:], scalar2=0.0,
            op0=ALU.is_ge, op1=ALU.add, accum_out=c1f[:],
        )
        nc.vector.tensor_scalar(
            out=kb[:], in0=s[:], scalar1=tau2[:], scalar2=0.0,
            op0=ALU.is_ge, op1=ALU.add, accum_out=c2f[:],
        )
        nc.vector.tensor_scalar(out=kc[:], in0=s[:], scalar1=lo[:], scalar2=None, op0=ALU.is_ge)
        # exclusive masks (overwrite kb, kc)
        nc.vector.tensor_tensor(out=kc[:], in0=kc[:], in1=kb[:], op=ALU.subtract)
        nc.vector.tensor_tensor(out=kb[:], in0=kb[:], in1=ka[:], op=ALU.subtract)
        # per-segment stable positions
        sa = wk.tile([128, N], F32, tag="sa")
        sb = wk.tile([128, N], F32, tag="sb")
        sc = wk.tile([128, N], F32, tag="sc")
        nc.vector.tensor_tensor_scan(out=sa[:], data0=ka[:], data1=ka[:], initial=0.0, op0=ALU.add, op1=ALU.bypass)
        nc.vector.tensor_tensor_scan(out=sb[:], data0=kb[:], data1=kb[:], initial=0.0, op0=ALU.add, op1=ALU.bypass)
        nc.vector.tensor_tensor_scan(out=sc[:], data0=kc[:], data1=kc[:], initial=0.0, op0=ALU.add, op1=ALU.bypass)
        # pos = ka*sa + kb*(sb+256) + kc*(sc+480) - 1, clamped to W-1
        nc.vector.tensor_tensor(out=sa[:], in0=sa[:], in1=ka[:], op=ALU.mult)
        nc.vector.scalar_tensor_tensor(out=sb[:], in0=sb[:], scalar=float(BASES[1]), in1=kb[:], op0=ALU.add, op1=ALU.mult)
        nc.vector.scalar_tensor_tensor(out=sc[:], in0=sc[:], scalar=float(BASES[2]), in1=kc[:], op0=ALU.add, op1=ALU.mult)
        nc.vector.tensor_tensor(out=sa[:], in0=sa[:], in1=sb[:], op=ALU.add)
        nc.vector.tensor_tensor(out=sa[:], in0=sa[:], in1=sc[:], op=ALU.add)
        nc.vector.tensor_scalar(
            out=sa[:], in0=sa[:], scalar1=1.0, scalar2=float(W - 1),
            op0=ALU.subtract, op1=ALU.min,
        )
        sidx = wk.tile([128, N], I16, tag="sidx", name=f"sidx_{t}", bufs=2)
        nc.vector.tensor_copy(sidx[:], sa[:])
        s16 = s[:].bitcast(U16).rearrange("p (n two) -> p n two", two=2)
        vlo = wk.tile([128, N], U16, tag="vlo", name=f"vlo_{t}", bufs=2)
        vhi = wk.tile([128, N], U16, tag="vhi", name=f"vhi_{t}", bufs=2)
        nc.vector.tensor_copy(vlo[:], s16[:, :, 0])
        nc.vector.tensor_copy(vhi[:], s16[:, :, 1])
        state[t] = (sidx, vlo, vhi, c1f, c2f)

    def stage_b(t):
        sidx, vlo, vhi, c1f, c2f = state[t]
        cmap = wk.tile([128, W], U16, tag="cmap", name=f"cmap_{t}", bufs=2)
        clo = wk.tile([128, W], U16, tag="clo", name=f"clo_{t}", bufs=2)
        chi = wk.tile([128, W], U16, tag="chi", name=f"chi_{t}", bufs=2)
        nc.gpsimd.local_scatter(cmap[:], iota_pos[:], sidx[:], channels=128, num_elems=W, num_idxs=N)
        nc.gpsimd.local_scatter(clo[:], vlo[:], sidx[:], channels=128, num_elems=W, num_idxs=N)
        nc.gpsimd.local_scatter(chi[:], vhi[:], sidx[:], channels=128, num_elems=W, num_idxs=N)
        state[t] = (cmap, clo, chi, c1f, c2f)

    def stage_c(t, seg):
        if seg == 0:
            cmap, clo, chi, c1f, c2f = state[t]
            comp = wk.tile([128, W], F32, tag="comp", name=f"comp_{t}")
            c16 = comp[:].bitcast(U16).rearrange("p (n two) -> p n two", two=2)
            nc.vector.tensor_copy(c16[:, :, 0], clo[:])
            nc.vector.tensor_copy(c16[:, :, 1], chi[:])
            mi = wk.tile([128, 8 * sum(ITERS)], U16, tag="mi", name=f"mi_{t}", bufs=2)
            state[t] = (cmap, chi, mi, c1f, c2f, comp)
        cmap, chi, mi, c1f, c2f, comp = state[t]
        mx = sm.tile([128, 8], F32, tag="mx")
        col = 8 * sum(ITERS[:seg])
        cc = comp[:, BASES[seg] : BASES[seg] + CAPS[seg]]
        for _ in range(ITERS[seg]):
            nc.vector.max(out=mx[:], in_=cc)
            nc.vector.max_index(out=mi[:, col : col + 8], in_max=mx[:], in_values=cc)
            nc.vector.match_replace(out=cc, in_to_replace=mx[:], in_values=cc, imm_value=NEG)
            col += 8

    def stage_d(t):
        cmap, chi, mi, c1f, c2f, _comp = state[t]
        # rank+1 data per segment (global rank = segment count offset + order)
        rk1 = wk.tile([128, W], U16, tag="rk1", name=f"rk1_{t}")
        col = 0
        for seg, (base, cap, its) in enumerate(zip(BASES, CAPS, ITERS)):
            ln = 8 * its
            if seg == 0:
                rdata = iota_rk1[:, :ln]
            else:
                off = c1f if seg == 1 else c2f
                rd = wk.tile([128, ln], U16, tag=f"rd{seg}", name=f"rd{seg}_{t}")
                nc.vector.tensor_scalar(out=rd[:], in0=iota_rk1[:, :ln], scalar1=off[:], scalar2=None, op0=ALU.add)
                rdata = rd[:]
            nc.gpsimd.local_scatter(
                rk1[:, base : base + cap], rdata, mi[:, col : col + ln].bitcast(I16),
                channels=128, num_elems=cap, num_idxs=ln,
            )
            col += 8 * its
        # kill garbage slots (zero-padding) and ranks > 512, then invert
        g = wk.tile([128, W], F32, tag="gmask", name=f"g_{t}")
        nc.vector.tensor_scalar(out=g[:], in0=chi[:], scalar1=0.5, scalar2=None, op0=ALU.is_gt)
        rf = wk.tile([128, W], F32, tag="rf", name=f"rf_{t}")
        nc.vector.tensor_tensor(out=rf[:], in0=rk1[:], in1=g[:], op=ALU.mult)
        nc.vector.tensor_scalar(out=g[:], in0=rf[:], scalar1=512.5, scalar2=None, op0=ALU.is_lt)
        nc.vector.tensor_tensor(out=rf[:], in0=rf[:], in1=g[:], op=ALU.mult)
        rk = wk.tile([128, W], I16, tag="rk", name=f"rk_{t}")
        nc.vector.tensor_scalar(out=rk[:], in0=rf[:], scalar1=1.0, scalar2=None, op0=ALU.subtract)
        oidx = wk.tile([128, K], U16, tag="oidx", name=f"oidx_{t}")
        nc.gpsimd.local_scatter(oidx[:], cmap[:], rk[:], channels=128, num_elems=K, num_idxs=W)
        nc.sync.dma_start(topk_d[128 * t : 128 * (t + 1), :], oidx[:])
        del state[t], scores[t]

    def make_pre(t):
        return {
            0: lambda: (stage_a(t), stage_b(t)),
            5: lambda: stage_c(t, 0),
            8: lambda: stage_c(t, 1),
            10: lambda: (stage_c(t, 2), stage_d(t)),
        }

    for t in range(NT):
        softmax(t, pre_ops=make_pre(t - 1) if t >= 1 else None)
    stage_a(NT - 1)
    stage_b(NT - 1)
    for seg in range(3):
        stage_c(NT - 1, seg)
    stage_d(NT - 1)



